# revision 51
# baseline (speedup 1.0000x reference)
"""Trainium2 Bass kernel for nn_DecoderBlock_Mamba (AxialDW conv + 1x1 conv +
BN + ReLU + LN + Mamba selective scan + residual).

Sharding: 8 cores = (batch b in 0..3) x (state-half sigma in {0,1}).
Each core runs the full per-image pipeline for its batch element but only 8 of
the 16 SSM states; partial y is AllReduce'd within core pairs, post-stack is
computed redundantly on both cores of a pair.

Self-contained: hardcodes all shapes; no sibling imports.
"""
import numpy as np

C = 64
DI = 128
DS = 16
DR = 4
B = 4
H = 64
W = 64
L = H * W
NS = 8            # states per core
NCORES = 8
ROW = W + 2       # padded row stride
LP = (H + 2) * ROW
NCH = 8           # L chunks of 512
CH = 512
EPS = 1e-5

_cached = {}


def _build_program(sim=False):
    import concourse.bass as bass
    import concourse.bacc as bacc
    import concourse.mybir as mybir
    import concourse.tile as tile

    dt = mybir.dt
    f32 = dt.float32
    bf16 = dt.bfloat16
    Act = mybir.ActivationFunctionType
    Alu = mybir.AluOpType
    Axis = mybir.AxisListType

    nc = bacc.Bacc(None, target_bir_lowering=False)

    def din(name, shape, dtype=f32):
        return nc.dram_tensor(name, shape, dtype, kind="ExternalInput")

    ximgs_d = din("ximgs", [C, 5 * L], bf16)
    cf32_d = din("cf32", [128, 19])
    cbf_d = din("cbf", [128, 2948], bf16)

    out_d = nc.dram_tensor("out_f", [C, L], f32, kind="ExternalOutput")

    groups = [[0, 1], [2, 3], [4, 5], [6, 7]]

    with tile.TileContext(nc) as tc:
        with (
            tc.tile_pool(name="dram", bufs=1, space="DRAM") as dpool,
            tc.tile_pool(name="const", bufs=1) as cpool,
            tc.tile_pool(name="big", bufs=1) as bpool,
            tc.tile_pool(name="sm", bufs=2) as spool,
            tc.tile_pool(name="da", bufs=2) as dapool,
            tc.tile_pool(name="dbx", bufs=2) as dbxpool,
            tc.tile_pool(name="ps", bufs=4, space="PSUM") as ps,
            tc.tile_pool(name="psy", bufs=2, space="PSUM") as psy,
        ):
            # ---- load constants (packed: 3 DMAs total) ----
            cf = cpool.tile([128, 19], f32)
            cb = cpool.tile([128, 2948], bf16)
            nc.sync.dma_start(cf[:], cf32_d[:])
            nc.sync.dma_start(cb[:], cbf_d[:])
            bn_s = cf[0:C, 0:1]
            bn_b = cf[0:C, 1:2]
            ip_b = cf[:, 2:4]
            cd_w = cf[:, 4:8]
            cd_b = cf[:, 8:9]
            dt_b = cf[:, 9:10]
            a_sc = cf[:, 10:18]
            Dp = cf[:, 18:19]
            ident = cb[:, 0:128]
            cw = cb[0:C, 128:448]
            ip_lhsT = cb[0:C, 448:704]
            xpdt_lhsT = cb[:, 704:708]
            dt_lhsT = cb[0:DR, 708:836]
            brep_lhsT = cb[:, 836:1860]
            crep_lhsT = cb[:, 1860:2884]
            op_lhsT = cb[:, 2884:2948]

            # ---- persistent activations ----
            SEQ = bpool.tile([C, L], bf16)           # BN+ReLU output (residual)
            HN = bpool.tile([C, L], bf16)            # LN-normalized (no affine)
            XM0 = bpool.tile([DI, L + 4], bf16)      # conv1d input, data @ col 4
            ZS = bpool.tile([DI, L], bf16)           # silu(z)
            XC = bpool.tile([DI, L], bf16)
            DT = bpool.tile([DI, L], bf16)
            U = bpool.tile([DI, L], bf16)
            Hs = [bpool.tile([DI, L], bf16, name=f"H{j}", tag=f"H{j}") for j in range(NS)]
            YSUM = bpool.tile([DI, L], bf16, name="YSUM", tag="U")

            # Prime ACT's vector clock on the const DMAs so later
            # activations (limited wait slots) don't re-wait on them.
            warm = cpool.tile([128, 1], f32, tag="warm")
            nc.scalar.activation(warm[:], cf[:, 0:1], Act.Copy)
            warm2 = cpool.tile([128, 1], bf16, tag="warm2")
            nc.scalar.activation(warm2[:], cb[:, 0:1], Act.Copy)
            eps_t = cpool.tile([128, 1], f32, tag="epsl")
            nc.gpsimd.memset(eps_t[:], EPS)
            nc.vector.tensor_scalar_mul(XM0[:, 0:4], cf[:, 0:4], 0.0)

            IMGS = [bpool.tile([C, L], bf16, name=f"img{t}", tag=f"H{t}")
                    for t in range(5)]
            for t in range(5):
                nc.sync.dma_start(IMGS[t][:], ximgs_d[:, t * L:(t + 1) * L])

            # ---- front conv: 5 accumulating taps + BN + ReLU ----
            for chi in range(NCH):
                sl = slice(chi * CH, (chi + 1) * CH)
                pc = ps.tile([C, CH], f32, tag="mm")
                for tap in range(5):
                    nc.tensor.matmul(pc[:], cw[:, tap * C:(tap + 1) * C],
                                     IMGS[tap][:, sl],
                                     start=(tap == 0), stop=(tap == 4))
                nc.scalar.activation(SEQ[:, chi * CH:(chi + 1) * CH], pc[:],
                                     Act.Relu, bias=bn_b, scale=bn_s)

            # ---- LayerNorm over channels, per 128-token block ----
            # Pass A: transpose + center, collect per-block variance columns.
            HN0 = bpool.tile([128, L // 2], bf16, name="HN0", tag="HN0")
            VARS = spool.tile([128, 32], f32, tag="VARS")
            for blk in range(L // 128):
                sl = slice(blk * 128, (blk + 1) * 128)
                tps = ps.tile([128, C], bf16, tag="mm")
                nc.tensor.transpose(tps[:], SEQ[:, sl], ident[0:C, 0:C])
                mu = spool.tile([128, 1], f32, tag="mu")
                nc.vector.tensor_reduce(mu[:], tps[:], Axis.X, Alu.add)
                mun = spool.tile([128, 1], f32, tag="mun")
                nc.vector.tensor_scalar_mul(mun[:], mu[:], 1.0 / C)
                h0 = HN0[:, blk * C:(blk + 1) * C]
                nc.vector.tensor_scalar_sub(h0, tps[:], mun[:])
                sqj = spool.tile([128, C], f32, tag="sqj")
                nc.vector.tensor_mul(sqj[:], h0, h0)
                ssq = spool.tile([128, 1], f32, tag="ssq")
                nc.vector.tensor_reduce(ssq[:], sqj[:], Axis.X, Alu.add)
                nc.vector.tensor_scalar(VARS[:, blk:blk + 1], ssq[:], 1.0 / C,
                                        EPS, op0=Alu.mult, op1=Alu.add)
            # Pass B: one sqrt + one reciprocal for all blocks.
            SQV = spool.tile([128, 32], f32, tag="SQV")
            nc.scalar.activation(SQV[:], VARS[:], Act.Sqrt)
            RSTD = spool.tile([128, 32], f32, tag="RSTD")
            nc.vector.reciprocal(RSTD[:], SQV[:])
            # Pass C: scale + transpose back (hnT slices of one tensor so
            # the 1-wait-slot tensor_scalar never sees slot-rotation deps).
            HNT = bpool.tile([128, L // 2], bf16, name="HNT", tag="HNT")
            for blk in range(L // 128):
                sl = slice(blk * 128, (blk + 1) * 128)
                hnT = HNT[:, blk * C:(blk + 1) * C]
                nc.vector.tensor_scalar_mul(hnT, HN0[:, blk * C:(blk + 1) * C],
                                            RSTD[:, blk:blk + 1])
                tb = ps.tile([C, 128], bf16, tag="mm")
                nc.tensor.transpose(tb[:], hnT, ident)
                nc.scalar.activation(HN[:, sl], tb[:], Act.Copy)

            # ---- in_proj ----
            for chi in range(NCH):
                sl = slice(chi * CH, (chi + 1) * CH)
                xm_ps = ps.tile([DI, CH], f32, tag="mm")
                z_ps = ps.tile([DI, CH], f32, tag="mm")
                nc.tensor.matmul(xm_ps[:], ip_lhsT[0:C, 0:DI], HN[:, sl],
                                 start=True, stop=True)
                nc.tensor.matmul(z_ps[:], ip_lhsT[0:C, DI:2 * DI], HN[:, sl],
                                 start=True, stop=True)
                nc.scalar.activation(XM0[:, 4 + chi * CH:4 + (chi + 1) * CH],
                                     xm_ps[:], Act.Identity, bias=ip_b[:, 0:1])
                nc.scalar.activation(ZS[:, sl], z_ps[:], Act.Silu,
                                     bias=ip_b[:, 1:2])
            # ---- causal conv1d (4 taps) + silu ----
            # xc_t = sum_k w_k * xm_{t-3+k}; XM0 holds xm at col 4,
            # XM1 at col 3: tap k reads XM0[1+k:] or XM1[k:] — use whichever
            # start offset is even so bf16 ops keep 4B alignment.
            ACC1 = bpool.tile([DI, L], bf16, name="ACC1", tag="ACC1")
            ACC2 = bpool.tile([DI, L], bf16, name="ACC2", tag="ACC2")
            nc.vector.tensor_scalar_mul(ACC1[:], XM0[:, 1:1 + L], cd_w[:, 0:1])
            nc.vector.scalar_tensor_tensor(ACC2[:], XM0[:, 2:2 + L], cd_w[:, 1:2],
                                           ACC1[:], op0=Alu.mult, op1=Alu.add)
            nc.vector.scalar_tensor_tensor(ACC1[:], XM0[:, 3:3 + L], cd_w[:, 2:3],
                                           ACC2[:], op0=Alu.mult, op1=Alu.add)
            nc.vector.scalar_tensor_tensor(ACC2[:], XM0[:, 4:4 + L], cd_w[:, 3:4],
                                           ACC1[:], op0=Alu.mult, op1=Alu.add)
            nc.scalar.activation(XC[:], ACC2[:], Act.Silu, bias=cd_b)

            # ---- x_proj (dt rows) + dt_proj + softplus ----
            for chi in range(NCH):
                sl = slice(chi * CH, (chi + 1) * CH)
                dtr_ps = ps.tile([DR, CH], f32, tag="mm")
                nc.tensor.matmul(dtr_ps[:], xpdt_lhsT, XC[:, sl],
                                 start=True, stop=True)
                dtr_sb = spool.tile([DR, CH], bf16, tag="dtrsb")
                nc.scalar.activation(dtr_sb[:], dtr_ps[:], Act.Copy)
                dt_ps = ps.tile([DI, CH], f32, tag="mm")
                nc.tensor.matmul(dt_ps[:], dt_lhsT, dtr_sb[:],
                                 start=True, stop=True)
                esb = spool.tile([DI, CH], f32, tag="esb")
                nc.scalar.activation(esb[:], dt_ps[:], Act.Exp, bias=dt_b)
                nc.scalar.activation(DT[:, sl], esb[:], Act.Ln, bias=1.0)
            nc.vector.tensor_mul(U[:], DT[:], XC[:])

            # ---- per-state: dA = exp(a_j*dt), dBx = u*B_j, scan ----
            LH = L // 2
            for j in range(NS):
                for half in range(2):
                    hsl = slice(half * LH, (half + 1) * LH)
                    dA = dapool.tile([DI, LH], f32, tag="dA")
                    nc.scalar.activation(dA[:], DT[:, hsl], Act.Exp,
                                         scale=a_sc[:, j:j + 1])
                    dbx = dbxpool.tile([DI, LH], bf16, tag="dbx")
                    for ci in range(LH // CH):
                        sl = slice(half * LH + ci * CH,
                                   half * LH + (ci + 1) * CH)
                        lsl = slice(ci * CH, (ci + 1) * CH)
                        br = ps.tile([DI, CH], f32, tag="mm")
                        nc.tensor.matmul(br[:], brep_lhsT[:, j * DI:(j + 1) * DI],
                                         XC[:, sl], start=True, stop=True)
                        brs = spool.tile([DI, CH], bf16, tag="brs")
                        nc.scalar.activation(brs[:], br[:], Act.Copy)
                        nc.gpsimd.tensor_tensor(dbx[:, lsl], U[:, sl], brs[:],
                                                op=Alu.mult)
                    init = 0.0 if half == 0 else Hs[j][:, LH - 1:LH]
                    nc.vector.tensor_tensor_scan(Hs[j][:, hsl], dA[:], dbx[:],
                                                 init, op0=Alu.mult, op1=Alu.add)

            # ---- y accumulation: y = sum_j H_j * C_j  (PE-accumulated) ----
            y_in_t = dpool.tile([DI, L], bf16, tag="yin")
            y_out_t = dpool.tile([DI, L], bf16, tag="yout")
            for chi in range(NCH):
                sl = slice(chi * CH, (chi + 1) * CH)
                yps = psy.tile([DI, CH], f32, tag="yps")
                for j in range(NS):
                    cr = ps.tile([DI, CH], f32, tag="mm")
                    nc.tensor.matmul(cr[:], crep_lhsT[:, j * DI:(j + 1) * DI],
                                     XC[:, sl], start=True, stop=True)
                    tmp = spool.tile([DI, CH], bf16, tag="ymul")
                    nc.vector.tensor_tensor(tmp[:], Hs[j][:, sl], cr[:],
                                            op=Alu.mult)
                    nc.tensor.matmul(yps[:], ident, tmp[:],
                                     start=(j == 0), stop=(j == NS - 1))
                ysb = spool.tile([DI, CH], bf16, tag="ysb")
                nc.scalar.activation(ysb[:], yps[:], Act.Copy)
                nc.sync.dma_start(y_in_t[:, sl], ysb[:])

            # ---- AllReduce partial y within batch pair ----
            if sim:
                nc.sync.dma_start(y_out_t[:], y_in_t[:])
            else:
                nc.gpsimd.collective_compute(
                    "AllReduce", Alu.add, replica_groups=groups,
                    ins=[y_in_t.opt()], outs=[y_out_t.opt()])
            nc.sync.dma_start(YSUM[:], y_out_t[:])

            # ---- post: ys = (y + xc*Dp) * silu(z); out = op(ys) + seq ----
            XCD = bpool.tile([DI, L], bf16, name="XCD", tag="DT")
            nc.vector.tensor_scalar_mul(XCD[:], XC[:], Dp)
            nc.vector.tensor_add(XCD[:], YSUM[:], XCD[:])
            YS = bpool.tile([DI, L], bf16, tag="YS")
            nc.vector.tensor_mul(YS[:], XCD[:], ZS[:])
            OUT = bpool.tile([C, L], f32, name="OUT", tag="XM0")
            for chi in range(NCH):
                sl = slice(chi * CH, (chi + 1) * CH)
                op_ps = ps.tile([C, CH], f32, tag="mm")
                nc.tensor.matmul(op_ps[:], op_lhsT, YS[:, sl],
                                 start=True, stop=True)
                nc.vector.tensor_tensor(OUT[:, sl], op_ps[:], SEQ[:, sl],
                                        op=Alu.add)
                nc.sync.dma_start(out_d[:, sl], OUT[:, sl])

    nc.compile()
    return nc


def _host_precompute(inp):
    import ml_dtypes
    f = lambda k: np.asarray(inp[k], np.float32)
    bf = lambda a: np.ascontiguousarray(a.astype(ml_dtypes.bfloat16))
    w1 = f("conv_w")[:, :, 0, 0]
    wh = f("dwh_w")[:, 0, :, 0]
    ww = f("dww_w")[:, 0, 0, :]
    taps = [
        w1 * (1.0 + wh[:, 1] + ww[:, 1])[None, :],   # center
        w1 * wh[:, 0][None, :],                       # up
        w1 * wh[:, 2][None, :],                       # down
        w1 * ww[:, 0][None, :],                       # left
        w1 * ww[:, 2][None, :],                       # right
    ]
    cw = np.concatenate([t.T for t in taps], axis=1)  # [cin=64, 5*64]
    btot = f("conv_b") + w1 @ (f("dwh_b") + f("dww_b"))
    s_bn = f("bn_g") / np.sqrt(f("bn_v") + EPS)
    bn_bias = s_bn * (btot - f("bn_m")) + f("bn_b")
    ipw = f("in_proj_w")
    ip_lhsT = (ipw * f("ln_g")[None, :]).T            # [64, 256]
    ip_bias = ipw @ f("ln_b")                          # [256]
    xpw = f("x_proj_w")                                # [36, 128]
    a_full = -np.exp(np.asarray(inp["A_log"], np.float32))  # [DI, DS]

    per_sigma = []
    for sg in range(2):
        s_lo = sg * NS
        cf32 = np.zeros((128, 19), np.float32)
        cf32[:C, 0] = s_bn
        cf32[:C, 1] = bn_bias
        cf32[:, 2] = ip_bias[:DI]
        cf32[:, 3] = ip_bias[DI:]
        cf32[:, 4:8] = f("convd_w")[:, 0, :]
        cf32[:, 8] = f("convd_b")
        cf32[:, 9] = f("dt_proj_b")
        for j in range(NS):
            cf32[:, 10 + j] = a_full[:, s_lo + j]
        cf32[:, 18] = f("Dp")

        cbf = np.zeros((128, 2948), np.float32)
        cbf[:, 0:128] = np.eye(128, dtype=np.float32)
        cbf[:C, 128:448] = cw
        cbf[:C, 448:704] = ip_lhsT
        cbf[:, 704:708] = xpw[:DR].T
        cbf[:DR, 708:836] = f("dt_proj_w").T
        for j in range(NS):
            s = s_lo + j
            cbf[:, 836 + j * DI:836 + (j + 1) * DI] = xpw[DR + s][:, None]
            cbf[:, 1860 + j * DI:1860 + (j + 1) * DI] = xpw[DR + DS + s][:, None]
        cbf[:, 2884:2948] = f("out_proj_w").T
        per_sigma.append(dict(cf32=cf32, cbf=bf(cbf)))
    return {}, per_sigma


def _shift_images(xb):
    # 5 pre-shifted copies: ctr, up(reads h-1), dn(h+1), lf(w-1), rt(w+1)
    import ml_dtypes
    out = np.zeros((C, 5, H, W), np.float32)
    out[:, 0] = xb
    out[:, 1, 1:, :] = xb[:, :-1, :]
    out[:, 2, :-1, :] = xb[:, 1:, :]
    out[:, 3, :, 1:] = xb[:, :, :-1]
    out[:, 4, :, :-1] = xb[:, :, 1:]
    return np.ascontiguousarray(
        out.transpose(1, 0, 2, 3).reshape(5, C, L).transpose(1, 0, 2)
        .reshape(C, 5 * L).astype(ml_dtypes.bfloat16))


TRACE = False
LAST_EXEC_NS = None
LAST_TRACE_DIR = None


def kernel(**inputs):
    global LAST_EXEC_NS, LAST_TRACE_DIR
    from concourse.bass_utils import run_bass_kernel_spmd

    if "nc" not in _cached:
        _cached["nc"] = _build_program()
    nc = _cached["nc"]

    common, per_sigma = _host_precompute(inputs)
    x = np.asarray(inputs["x"], np.float32)
    in_maps = []
    for c in range(NCORES):
        b, sg = c // 2, c % 2
        m = dict(common)
        m.update(per_sigma[sg])
        m["ximgs"] = _shift_images(x[b])
        in_maps.append(m)

    kw = {}
    if TRACE:
        import tempfile
        LAST_TRACE_DIR = tempfile.mkdtemp(prefix="bass_trace_")
        kw = dict(trace=True, tmpdir=LAST_TRACE_DIR)
    r = run_bass_kernel_spmd(nc, in_maps, list(range(NCORES)), **kw)
    if r.exec_time_ns is not None:
        LAST_EXEC_NS = r.exec_time_ns
    res = r.results
    out = np.empty((B, C, H, W), np.float32)
    for b in range(B):
        out[b] = np.asarray(res[2 * b]["out_f"], np.float32).reshape(C, H, W)
    return out
